# revision 1
# baseline (speedup 1.0000x reference)
"""Local sliding-window attention block (MQA + partial RoPE) on 8 TRN2 cores.

Sharding: 2 batches x 4 sequence chunks of 512 queries each. Each core
computes q/k/v projections for its chunk (keys include a 512-token halo),
windowed attention (window=512, causal), and the o-projection for its own
query rows — so the host-side unshard is a pure concatenation.

On-chip layout: everything transposed (feature dim on partitions).
  xT[d, pos]  ->  Q^T[dh, q] / K^T[dh, k] (RoPE'd)  ->  S^T[k, q]
  -> exp -> P^T[k, q] (bf16, multiplicative 0/1 masks)
  -> O^T[dv, q] = V.T-matmul  -> normalized by softmax denominators
     (partition_all_reduce on GPSIMD)  -> used directly as lhsT of o-proj.
All matmuls bf16 inputs, fp32 PSUM accumulation.
"""

import numpy as np
import ml_dtypes

BF16 = ml_dtypes.bfloat16

B, L, D = 2, 2048, 2048
H, HD = 16, 128
ROPE_DIMS, HALF = 64, 32
WINDOW = 512
ROPE_BASE = 10000.0
SCALE = HD ** -0.5

CHUNK = 512            # queries per core
NK = 1024              # keys (incl. halo) per core
NQT = CHUNK // 128     # 4 local query tiles
NKT = NK // 128        # 8 local key tiles
NSIG = 5               # key tiles in window per query tile
NDT = D // 128         # 16 contraction tiles over embedding dim

_PROGRAM = None


def _rope(nc, mybir, pool_tmp, out_bf, ps, cos2, sin2m, fp32):
    """out[0:64] = rotary(ps[0:64]); out[64:128] = ps[64:128]. ps fp32, out bf16.

    cos2 is [64, n] with rows [0:32]==[32:64]==cos(theta); sin2m has rows
    [0:32]==-sin(theta), [32:64]==+sin(theta). Engines can't read across
    partitions, so the half-swap (x2 into rows 0:32, x1 into rows 32:64)
    goes through two partition-shifting DMAs; then
      out[0:64] = ps[0:64]*cos2 + swapped*sin2m
    is partition-aligned elementwise math.
    """
    n = cos2.shape[-1]
    sb64 = pool_tmp.tile([ROPE_DIMS, n], fp32, tag="rope_sb64")
    nc.scalar.copy(sb64, ps[0:ROPE_DIMS])
    ss = pool_tmp.tile([ROPE_DIMS, n], fp32, tag="rope_ss")
    nc.sync.dma_start(out=ss[0:HALF], in_=sb64[HALF:ROPE_DIMS])
    nc.sync.dma_start(out=ss[HALF:ROPE_DIMS], in_=sb64[0:HALF])
    mcos = pool_tmp.tile([ROPE_DIMS, n], fp32, tag="rope_mcos")
    nc.vector.tensor_mul(mcos, ps[0:ROPE_DIMS], cos2)
    nc.vector.tensor_mul(ss, ss, sin2m)
    nc.vector.tensor_add(out_bf[0:ROPE_DIMS], mcos, ss)
    nc.scalar.copy(out_bf[ROPE_DIMS:HD], ps[ROPE_DIMS:HD])


def _build_program():
    from contextlib import ExitStack
    import concourse.bass as bass
    import concourse.mybir as mybir
    import concourse.tile as tile
    import concourse.bass_isa as bass_isa
    from concourse import bacc

    fp32 = mybir.dt.float32
    bf16 = mybir.dt.bfloat16
    AF = mybir.ActivationFunctionType

    nc = bacc.Bacc(None, target_bir_lowering=False)

    xT_d = nc.dram_tensor("xT", [D, NK], bf16, kind="ExternalInput")
    wq_d = nc.dram_tensor("Wq", [D, D], bf16, kind="ExternalInput")
    wk_d = nc.dram_tensor("Wk", [D, HD], bf16, kind="ExternalInput")
    wv_d = nc.dram_tensor("Wv", [D, HD], bf16, kind="ExternalInput")
    wo_d = nc.dram_tensor("Wo", [D, D], bf16, kind="ExternalInput")
    bo_d = nc.dram_tensor("bo", [1, D], fp32, kind="ExternalInput")
    cos_d = nc.dram_tensor("cosT", [ROPE_DIMS, NK], fp32, kind="ExternalInput")
    sin_d = nc.dram_tensor("sinT", [ROPE_DIMS, NK], fp32, kind="ExternalInput")
    msk_d = nc.dram_tensor("masks", [NQT, NSIG, 128, 128], bf16, kind="ExternalInput")
    out_d = nc.dram_tensor("out", [CHUNK, D], fp32, kind="ExternalOutput")

    with tile.TileContext(nc) as tc, ExitStack() as ctx:
        p_const = ctx.enter_context(tc.tile_pool(name="const", bufs=1))
        p_xt = ctx.enter_context(tc.tile_pool(name="xt", bufs=1))
        p_kv = ctx.enter_context(tc.tile_pool(name="kv", bufs=1))
        p_wq = ctx.enter_context(tc.tile_pool(name="wq", bufs=3))
        p_qt = ctx.enter_context(tc.tile_pool(name="qt", bufs=3))
        p_es = ctx.enter_context(tc.tile_pool(name="es", bufs=6))
        p_red = ctx.enter_context(tc.tile_pool(name="red", bufs=4))
        p_tmp = ctx.enter_context(tc.tile_pool(name="tmp", bufs=4))
        p_otn = ctx.enter_context(tc.tile_pool(name="otn", bufs=1))
        p_wo = ctx.enter_context(tc.tile_pool(name="wo", bufs=3))
        p_ob = ctx.enter_context(tc.tile_pool(name="ob", bufs=4))

        # ---- persistent loads ----
        xview = xT_d[:].rearrange("(n p) m -> n p m", p=128)
        xt = []
        for i in range(NDT):
            t_ = p_xt.tile([128, NK], bf16, tag=f"xt{i}")
            nc.sync.dma_start(out=t_, in_=xview[i])
            xt.append(t_)

        wk_sb = p_const.tile([128, NDT, HD], bf16, tag="wk")
        nc.sync.dma_start(out=wk_sb, in_=wk_d[:].rearrange("(n p) m -> p n m", p=128))
        wv_sb = p_const.tile([128, NDT, HD], bf16, tag="wv")
        nc.sync.dma_start(out=wv_sb, in_=wv_d[:].rearrange("(n p) m -> p n m", p=128))

        cos_sb = p_const.tile([ROPE_DIMS, NK], fp32, tag="cos")
        nc.sync.dma_start(out=cos_sb, in_=cos_d[:])
        sin_sb = p_const.tile([ROPE_DIMS, NK], fp32, tag="sin")
        nc.sync.dma_start(out=sin_sb, in_=sin_d[:])

        msk_sb = p_const.tile([128, NQT, NSIG, 128], bf16, tag="msk")
        nc.sync.dma_start(out=msk_sb, in_=msk_d[:].rearrange("t s k q -> k t s q"))

        bias_sb = p_const.tile([128, D], fp32, tag="bias")
        nc.sync.dma_start(
            out=bias_sb, in_=bass.AP(tensor=bo_d, offset=0, ap=[[0, 128], [1, D]])
        )

        # ---- K^T (RoPE'd) and V projections ----
        kt = p_kv.tile([128, NK], bf16, tag="kt")
        v_sb = []
        for s in range(NKT):
            t_ = p_kv.tile([128, HD], bf16, tag=f"v{s}")
            v_sb.append(t_)

        with tc.tile_pool(name="ps_kv", bufs=2, space=bass.MemorySpace.PSUM) as ps_kv:
            for nh in range(NK // 512):
                ps = ps_kv.tile([128, 512], fp32, tag="ps_kv")
                cols = slice(nh * 512, (nh + 1) * 512)
                for dt in range(NDT):
                    nc.tensor.matmul(
                        ps, wk_sb[:, dt, :], xt[dt][:, cols],
                        start=(dt == 0), stop=(dt == NDT - 1),
                    )
                _rope(nc, mybir, p_tmp, kt[:, cols], ps,
                      cos_sb[:, cols], sin_sb[:, cols], fp32)

            for s in range(NKT):
                psv = ps_kv.tile([128, HD], fp32, tag="ps_v")
                cols = slice(s * 128, (s + 1) * 128)
                for dt in range(NDT):
                    nc.tensor.matmul(
                        psv, xt[dt][:, cols], wv_sb[:, dt, :],
                        start=(dt == 0), stop=(dt == NDT - 1),
                    )
                nc.scalar.copy(v_sb[s], psv)

        # ---- per-head attention ----
        otn = []
        for h in range(H):
            t_ = p_otn.tile([128, CHUNK], bf16, tag=f"otn{h}")
            otn.append(t_)

        wqview = wq_d[:].rearrange("(n p) m -> p n m", p=128)
        with (
            tc.tile_pool(name="ps_q", bufs=2, space=bass.MemorySpace.PSUM) as ps_qp,
            tc.tile_pool(name="ps_s", bufs=2, space=bass.MemorySpace.PSUM) as ps_sp,
            tc.tile_pool(name="ps_o", bufs=2, space=bass.MemorySpace.PSUM) as ps_op,
        ):
            for h in range(H):
                wq_h = p_wq.tile([128, NDT, 128], bf16, tag="wq")
                nc.sync.dma_start(out=wq_h, in_=wqview[:, :, h * 128:(h + 1) * 128])
                psq = ps_qp.tile([128, CHUNK], fp32, tag="ps_q")
                for dt in range(NDT):
                    nc.tensor.matmul(
                        psq, wq_h[:, dt, :], xt[dt][:, CHUNK:NK],
                        start=(dt == 0), stop=(dt == NDT - 1),
                    )
                qt = p_qt.tile([128, CHUNK], bf16, tag="qt")
                _rope(nc, mybir, p_tmp, qt, psq,
                      cos_sb[:, CHUNK:NK], sin_sb[:, CHUNK:NK], fp32)

                otp = ps_op.tile([128, CHUNK], fp32, tag="ps_o")
                recip = p_red.tile([128, NQT, 128], fp32, tag="recip")
                for t in range(NQT):
                    pss = ps_sp.tile([128, NSIG, 128], fp32, tag="ps_s")
                    qsl = qt[:, t * 128:(t + 1) * 128]
                    for sig in range(NSIG):
                        s = t + sig
                        nc.tensor.matmul(
                            pss[:, sig, :], kt[:, s * 128:(s + 1) * 128], qsl,
                            start=True, stop=True,
                        )
                    es = p_es.tile([128, NSIG, 128], bf16, tag="es")
                    nc.scalar.activation(es, pss, AF.Exp, scale=SCALE)
                    nc.vector.tensor_mul(es, es, msk_sb[:, t, :, :])
                    red = p_red.tile([128, 128], fp32, tag="red")
                    nc.vector.reduce_sum(
                        out=red, in_=es.rearrange("p s q -> p q s"),
                        axis=mybir.AxisListType.X,
                    )
                    nc.gpsimd.partition_all_reduce(
                        recip[:, t, :], red, channels=128,
                        reduce_op=bass_isa.ReduceOp.add,
                    )
                    for sig in range(NSIG):
                        nc.tensor.matmul(
                            otp[:, t * 128:(t + 1) * 128],
                            v_sb[t + sig], es[:, sig, :],
                            start=(sig == 0), stop=(sig == NSIG - 1),
                        )
                rview = recip.rearrange("p t q -> p (t q)")
                nc.vector.reciprocal(rview, rview)
                nc.vector.tensor_mul(otn[h], otp, rview)

        # ---- o-projection + bias ----
        woview = wo_d[:].rearrange("(h p) m -> p h m", p=128)
        with tc.tile_pool(name="ps_out", bufs=4, space=bass.MemorySpace.PSUM) as ps_outp:
            for n in range(D // 512):
                wo_n = p_wo.tile([128, H, 512], bf16, tag="wo")
                nc.sync.dma_start(out=wo_n, in_=woview[:, :, n * 512:(n + 1) * 512])
                for t in range(NQT):
                    pso = ps_outp.tile([128, 512], fp32, tag="ps_out")
                    for h in range(H):
                        nc.tensor.matmul(
                            pso, otn[h][:, t * 128:(t + 1) * 128], wo_n[:, h, :],
                            start=(h == 0), stop=(h == H - 1),
                        )
                    ob = p_ob.tile([128, 512], fp32, tag="ob")
                    nc.vector.tensor_add(ob, pso, bias_sb[:, n * 512:(n + 1) * 512])
                    nc.sync.dma_start(
                        out=out_d[t * 128:(t + 1) * 128, n * 512:(n + 1) * 512],
                        in_=ob,
                    )

    nc.compile()
    return nc


def _get_program():
    global _PROGRAM
    if _PROGRAM is None:
        _PROGRAM = _build_program()
    return _PROGRAM


def _make_in_maps(x, Wq, Wk, Wv, Wo, bo):
    Wq_b = np.ascontiguousarray(Wq.astype(BF16))
    Wk_b = np.ascontiguousarray(Wk.astype(BF16))
    Wv_b = np.ascontiguousarray(Wv.astype(BF16))
    Wo_b = np.ascontiguousarray(Wo.astype(BF16))
    bo_f = np.ascontiguousarray(bo.astype(np.float32).reshape(1, D))

    inv_freq = np.exp(
        -np.log(np.float32(ROPE_BASE))
        * (np.arange(0, ROPE_DIMS, 2, dtype=np.float32) / np.float32(ROPE_DIMS))
    ).astype(np.float32)

    in_maps = []
    for c in range(8):
        b, g = divmod(c, 4)
        k_start = 512 * g - 512
        xs = np.zeros((NK, D), np.float32)
        lo = max(0, k_start)
        xs[lo - k_start:] = x[b, lo:k_start + NK]
        xT = np.ascontiguousarray(xs.T).astype(BF16)

        pos = (k_start + np.arange(NK)).astype(np.float32)
        theta = pos[None, :] * inv_freq[:, None]          # [32, NK]
        cos2 = np.ascontiguousarray(
            np.concatenate([np.cos(theta)] * 2, axis=0).astype(np.float32))
        sin2 = np.ascontiguousarray(
            np.concatenate([-np.sin(theta), np.sin(theta)], axis=0).astype(np.float32))

        m = np.zeros((NQT, NSIG, 128, 128), np.float32)
        for t in range(NQT):
            Tg = NQT * g + t
            for sig in range(NSIG):
                S = Tg - 4 + sig
                if S < 0:
                    continue
                i = (128 * Tg + np.arange(128))[None, :]   # queries (cols)
                j = (128 * S + np.arange(128))[:, None]    # keys (rows)
                m[t, sig] = (((i - j) >= 0) & ((i - j) < WINDOW)).astype(np.float32)
        masks = np.ascontiguousarray(m.astype(BF16))

        in_maps.append({
            "xT": xT, "Wq": Wq_b, "Wk": Wk_b, "Wv": Wv_b, "Wo": Wo_b,
            "bo": bo_f, "cosT": cos2, "sinT": sin2, "masks": masks,
        })
    return in_maps


def _unshard(results):
    out = np.zeros((B, L, D), np.float32)
    for c in range(8):
        b, g = divmod(c, 4)
        out[b, CHUNK * g:CHUNK * (g + 1)] = results[c]["out"]
    return out


def kernel(x, Wq, Wk, Wv, Wo, bo):
    from concourse.bass_utils import run_bass_kernel_spmd

    nc = _get_program()
    in_maps = _make_in_maps(x, Wq, Wk, Wv, Wo, bo)
    res = run_bass_kernel_spmd(nc, in_maps, core_ids=list(range(8)))
    return _unshard(res.results)



# revision 4
# speedup vs baseline: 1.2621x; 1.2621x over previous
"""Local sliding-window attention block (MQA + partial RoPE) on 8 TRN2 cores.

Sharding: 2 batches x 4 sequence chunks of 512 queries each. Each core
computes q/k/v projections for its chunk (keys include a 512-token halo),
windowed attention (window=512, causal), and the o-projection for its own
query rows — so the host-side unshard is a pure concatenation.

On-chip layout: everything transposed (feature dim on partitions).
  xT[d, pos]  ->  Q^T[dh, q] / K^T[dh, k] (RoPE'd)  ->  S^T[k, q]
  -> exp -> P^T[k, q] (bf16)  -> O^T[dv, q] = V.T-matmul.

Softmax plumbing (all off the GPSIMD/critical path):
  - scores grouped per KEY tile s: one matmul of N = 128*(#query tiles in
    window of s), so 8 S-matmuls per head instead of 20.
  - halo keys (cores at chunk 0) are killed with a per-partition bias AP on
    the Exp activation (exp(0*scale - 30) ~= 0), so no full masks needed;
    only the two window-edge triangles are multiplied in.
  - denominators: vector reduce over the 5 in-window key tiles per query
    tile (bf16), then a [128,16]-selector matmul packs sum_k into row h of
    ONE [16,512] PSUM tile accumulated across all 16 heads. One
    reciprocal_approx_fast on [16,512], one selector matmul per head
    broadcasts 1/d back to [128,512], and a vector multiply normalizes.
  - RoPE half-swap runs as a PE permutation matmul instead of SBUF DMAs.
All matmuls bf16 inputs, fp32 PSUM accumulation.
"""

import numpy as np
import ml_dtypes

BF16 = ml_dtypes.bfloat16

B, L, D = 2, 2048, 2048
H, HD = 16, 128
ROPE_DIMS, HALF = 64, 32
WINDOW = 512
ROPE_BASE = 10000.0
SCALE = HD ** -0.5

CHUNK = 512            # queries per core
NK = 1024              # keys (incl. halo) per core
NQT = CHUNK // 128     # 4 local query tiles
NKT = NK // 128        # 8 local key tiles
NDT = D // 128         # 16 contraction tiles over embedding dim

_PROGRAM = None


def _build_program():
    from contextlib import ExitStack
    import concourse.bass as bass
    import concourse.mybir as mybir
    import concourse.tile as tile
    from concourse import bacc

    fp32 = mybir.dt.float32
    bf16 = mybir.dt.bfloat16
    AF = mybir.ActivationFunctionType

    nc = bacc.Bacc(None, target_bir_lowering=False)

    xT_d = nc.dram_tensor("xT", [D, NK], bf16, kind="ExternalInput")
    wq_d = nc.dram_tensor("Wq", [H, 128, NDT, 128], bf16, kind="ExternalInput")
    wk_d = nc.dram_tensor("Wk", [128, NDT, HD], bf16, kind="ExternalInput")
    wv_d = nc.dram_tensor("Wv", [128, NDT, HD], bf16, kind="ExternalInput")
    wo_d = nc.dram_tensor("Wo", [4, 128, H, 512], bf16, kind="ExternalInput")
    bo_d = nc.dram_tensor("bo", [1, D], fp32, kind="ExternalInput")
    cos_d = nc.dram_tensor("cosT", [ROPE_DIMS, NK], fp32, kind="ExternalInput")
    sin_d = nc.dram_tensor("sinT", [ROPE_DIMS, NK], fp32, kind="ExternalInput")
    msk_d = nc.dram_tensor("masks", [128, NKT, 128], bf16, kind="ExternalInput")
    ebias_d = nc.dram_tensor("ebias", [128, NKT], fp32, kind="ExternalInput")
    esl_d = nc.dram_tensor("eslide", [128, 31], bf16, kind="ExternalInput")
    sel_d = nc.dram_tensor("sel16", [16, H, 128], bf16, kind="ExternalInput")
    prm_d = nc.dram_tensor("perm64", [ROPE_DIMS, ROPE_DIMS], bf16, kind="ExternalInput")
    out_d = nc.dram_tensor("out", [CHUNK, D], fp32, kind="ExternalOutput")

    # per key tile s, the query-column range it serves (absolute, in [0,512))
    def qcols(s):
        return max(s - 4, 0) * 128, min(s + 1, 4) * 128

    with tile.TileContext(nc) as tc, ExitStack() as ctx:
        p_const = ctx.enter_context(tc.tile_pool(name="const", bufs=1))
        p_xt = ctx.enter_context(tc.tile_pool(name="xt", bufs=1))
        p_kv = ctx.enter_context(tc.tile_pool(name="kv", bufs=1))
        p_wq = ctx.enter_context(tc.tile_pool(name="wq", bufs=3))
        p_qt = ctx.enter_context(tc.tile_pool(name="qt", bufs=3))
        p_es = ctx.enter_context(tc.tile_pool(name="es", bufs=2))
        p_red = ctx.enter_context(tc.tile_pool(name="red", bufs=4))
        p_tmp = ctx.enter_context(tc.tile_pool(name="tmp", bufs=2))
        p_otn = ctx.enter_context(tc.tile_pool(name="otn", bufs=1))
        p_wo = ctx.enter_context(tc.tile_pool(name="wo", bufs=2))
        p_ob = ctx.enter_context(tc.tile_pool(name="ob", bufs=4))
        ps_perm = ctx.enter_context(
            tc.tile_pool(name="ps_perm", bufs=1, space=bass.MemorySpace.PSUM)
        )

        # ---- persistent loads ----
        xview = xT_d[:].rearrange("(n p) m -> n p m", p=128)
        xt = []
        for i in range(NDT):
            t_ = p_xt.tile([128, NK], bf16, tag=f"xt{i}")
            nc.sync.dma_start(out=t_, in_=xview[i])
            xt.append(t_)

        wk_sb = p_const.tile([128, NDT, HD], bf16, tag="wk")
        nc.sync.dma_start(out=wk_sb, in_=wk_d[:])
        wv_sb = p_const.tile([128, NDT, HD], bf16, tag="wv")
        nc.sync.dma_start(out=wv_sb, in_=wv_d[:])

        cos_sb = p_const.tile([ROPE_DIMS, NK], fp32, tag="cos")
        nc.sync.dma_start(out=cos_sb, in_=cos_d[:])
        sin_sb = p_const.tile([ROPE_DIMS, NK], fp32, tag="sin")
        nc.sync.dma_start(out=sin_sb, in_=sin_d[:])

        msk_sb = p_const.tile([128, NKT, 128], bf16, tag="msk")
        nc.sync.dma_start(out=msk_sb, in_=msk_d[:])
        ebias_sb = p_const.tile([128, NKT], fp32, tag="ebias")
        nc.sync.dma_start(out=ebias_sb, in_=ebias_d[:])
        esl_sb = p_const.tile([128, 31], bf16, tag="eslide")
        nc.sync.dma_start(out=esl_sb, in_=esl_d[:])
        sel_sb = p_const.tile([16, H, 128], bf16, tag="sel16")
        nc.sync.dma_start(out=sel_sb, in_=sel_d[:])
        prm_sb = p_const.tile([ROPE_DIMS, ROPE_DIMS], bf16, tag="perm64")
        nc.sync.dma_start(out=prm_sb, in_=prm_d[:])

        bias_sb = p_const.tile([128, D], fp32, tag="bias")
        nc.sync.dma_start(
            out=bias_sb, in_=bass.AP(tensor=bo_d, offset=0, ap=[[0, 128], [1, D]])
        )

        def rope(out_bf, ps, cols):
            """out_bf[0:64] = rotary(ps[0:64]); out_bf[64:128] = ps[64:128].

            ps is fp32 PSUM [128, n]; the half-swap (x2 to rows 0:32, x1 to
            rows 32:64) runs as a PE permutation matmul; cos/sin tiles carry
            the duplicated/sign-flipped rows so the rest is partition-aligned
            vector math."""
            n = cols.stop - cols.start
            sb64 = p_tmp.tile([ROPE_DIMS, n], bf16, tag="rope_sb64")
            nc.scalar.copy(sb64, ps[0:ROPE_DIMS])
            psw = ps_perm.tile([ROPE_DIMS, n], fp32, tag="psw")
            nc.tensor.matmul(psw, prm_sb, sb64, start=True, stop=True)
            mcos = p_tmp.tile([ROPE_DIMS, n], fp32, tag="rope_mcos")
            nc.vector.tensor_mul(mcos, ps[0:ROPE_DIMS], cos_sb[:, cols])
            msin = p_tmp.tile([ROPE_DIMS, n], fp32, tag="rope_msin")
            nc.vector.tensor_mul(msin, psw, sin_sb[:, cols])
            nc.vector.tensor_add(out_bf[0:ROPE_DIMS], mcos, msin)
            nc.scalar.copy(out_bf[ROPE_DIMS:HD], ps[ROPE_DIMS:HD])

        # ---- K^T (RoPE'd) and V projections ----
        kt = p_kv.tile([128, NK], bf16, tag="kt")
        v_sb = []
        for s in range(NKT):
            t_ = p_kv.tile([128, HD], bf16, tag=f"v{s}")
            v_sb.append(t_)

        with tc.tile_pool(name="ps_kv", bufs=2, space=bass.MemorySpace.PSUM) as ps_kv:
            for nh in range(NK // 512):
                ps = ps_kv.tile([128, 512], fp32, tag="ps_kv")
                cols = slice(nh * 512, (nh + 1) * 512)
                for dt in range(NDT):
                    nc.tensor.matmul(
                        ps, wk_sb[:, dt, :], xt[dt][:, cols],
                        start=(dt == 0), stop=(dt == NDT - 1),
                    )
                rope(kt[:, cols], ps, cols)

            for s in range(NKT):
                psv = ps_kv.tile([128, HD], fp32, tag="ps_v")
                cols = slice(s * 128, (s + 1) * 128)
                for dt in range(NDT):
                    nc.tensor.matmul(
                        psv, xt[dt][:, cols], wv_sb[:, dt, :],
                        start=(dt == 0), stop=(dt == NDT - 1),
                    )
                nc.scalar.copy(v_sb[s], psv)

        # prefetch the first o-projection weight chunks during attention
        woview = wo_d[:]
        wo_n = []
        for n in range(2):
            t_ = p_wo.tile([128, H, 512], bf16, tag="wo")
            nc.sync.dma_start(out=t_, in_=woview[n])
            wo_n.append(t_)

        # ---- per-head attention ----
        otn = []
        for h in range(H):
            t_ = p_otn.tile([128, CHUNK], bf16, tag=f"otn{h}")
            otn.append(t_)

        with (
            tc.tile_pool(name="ps_q", bufs=2, space=bass.MemorySpace.PSUM) as ps_qp,
            tc.tile_pool(name="ps_s", bufs=2, space=bass.MemorySpace.PSUM) as ps_sp,
            tc.tile_pool(name="ps_o", bufs=2, space=bass.MemorySpace.PSUM) as ps_op,
            tc.tile_pool(name="ps_d", bufs=1, space=bass.MemorySpace.PSUM) as ps_dp,
        ):
            dall = ps_dp.tile([16, CHUNK], fp32, tag="dall")

            def qproj(h):
                wq_h = p_wq.tile([128, NDT, 128], bf16, tag="wq")
                nc.sync.dma_start(out=wq_h, in_=wq_d[h])
                psq = ps_qp.tile([128, CHUNK], fp32, tag="ps_q")
                for dt in range(NDT):
                    nc.tensor.matmul(
                        psq, wq_h[:, dt, :], xt[dt][:, CHUNK:NK],
                        start=(dt == 0), stop=(dt == NDT - 1),
                    )
                qt = p_qt.tile([128, CHUNK], bf16, tag="qt")
                rope(qt, psq, slice(CHUNK, NK))
                return qt

            qt = qproj(0)
            for h in range(H):
                # scores + exp + edge masks, grouped per key tile
                es = p_es.tile([128, NKT, 512], bf16, tag="es")
                for s in range(NKT):
                    lo, hi = qcols(s)
                    pss = ps_sp.tile([128, 512], fp32, tag="ps_s")
                    nc.tensor.matmul(
                        pss[:, 0:hi - lo],
                        kt[:, s * 128:(s + 1) * 128], qt[:, lo:hi],
                        start=True, stop=True,
                    )
                    nc.scalar.activation(
                        es[:, s, lo:hi], pss[:, 0:hi - lo], AF.Exp,
                        scale=SCALE, bias=ebias_sb[:, s:s + 1],
                    )
                    te = s if s <= 3 else s - 4
                    ecols = slice(te * 128, (te + 1) * 128)
                    nc.vector.tensor_mul(
                        es[:, s, ecols], es[:, s, ecols], msk_sb[:, s, :]
                    )

                # fill the PE while the reductions run: next head's q-proj
                if h + 1 < H:
                    qt = qproj(h + 1)

                # denominators: row h of dall accumulates sum_k per query.
                # One matmul per head into a full-bank group (start zeroes
                # the whole bank; sub-range starts would corrupt it).
                red = p_red.tile([128, NQT, 128], bf16, tag="red")
                for t in range(NQT):
                    with nc.allow_low_precision("softmax denom partial; rel err ~2^-9"):
                        nc.vector.reduce_sum(
                            out=red[:, t, :],
                            in_=es[:, t:t + 5, t * 128:(t + 1) * 128].rearrange(
                                "p s q -> p q s"
                            ),
                            axis=mybir.AxisListType.X,
                        )
                nc.tensor.matmul(
                    dall, esl_sb[:, 15 - h:31 - h],
                    red.rearrange("p t q -> p (t q)"),
                    start=(h == 0), stop=(h == H - 1),
                    skip_group_check=True,
                )

                # PV, grouped per key tile: one group per head; the s=0
                # start zeroes the whole otp bank, later tiles accumulate
                # their query-column ranges.
                otp = ps_op.tile([128, CHUNK], fp32, tag="ps_o")
                for s in range(NKT):
                    lo, hi = qcols(s)
                    nc.tensor.matmul(
                        otp[:, lo:hi], v_sb[s], es[:, s, lo:hi],
                        start=(s == 0), stop=(s == NKT - 1),
                        skip_group_check=True,
                    )
                nc.scalar.copy(otn[h], otp)

            # normalize: 1/d once on [16,512], broadcast per head via PE
            dsb = p_const.tile([16, CHUNK], fp32, tag="dsb")
            nc.scalar.copy(dsb, dall)
            rec_f = p_const.tile([16, CHUNK], fp32, tag="rec_f")
            nc.vector.reciprocal_approx_fast(out=rec_f, in_=dsb)
            rec_bf = p_const.tile([16, CHUNK], bf16, tag="rec_bf")
            nc.scalar.copy(rec_bf, rec_f)
            for h in range(H):
                ps_rec = ps_sp.tile([128, 512], fp32, tag="ps_s")
                nc.tensor.matmul(
                    ps_rec, sel_sb[:, h, :], rec_bf, start=True, stop=True
                )
                nc.vector.tensor_mul(otn[h], otn[h], ps_rec)

        # ---- o-projection + bias ----
        with tc.tile_pool(name="ps_out", bufs=4, space=bass.MemorySpace.PSUM) as ps_outp:
            for n in range(D // 512):
                if n >= 2:
                    wo_n[n % 2] = p_wo.tile(
                        [128, H, 512], bf16, tag="wo", name=f"wo{n}"
                    )
                    nc.sync.dma_start(out=wo_n[n % 2], in_=woview[n])
                wo_t = wo_n[n % 2]
                for t in range(NQT):
                    pso = ps_outp.tile([128, 512], fp32, tag="ps_out")
                    for h in range(H):
                        nc.tensor.matmul(
                            pso, otn[h][:, t * 128:(t + 1) * 128], wo_t[:, h, :],
                            start=(h == 0), stop=(h == H - 1),
                        )
                    ob = p_ob.tile([128, 512], fp32, tag="ob")
                    nc.vector.tensor_add(ob, pso, bias_sb[:, n * 512:(n + 1) * 512])
                    nc.sync.dma_start(
                        out=out_d[t * 128:(t + 1) * 128, n * 512:(n + 1) * 512],
                        in_=ob,
                    )

    nc.compile()
    return nc


def _get_program():
    global _PROGRAM
    if _PROGRAM is None:
        _PROGRAM = _build_program()
    return _PROGRAM


def _make_in_maps(x, Wq, Wk, Wv, Wo, bo):
    x = np.asarray(x, np.float32)
    # weights pre-arranged on host so every device DMA is contiguous
    Wq_b = np.ascontiguousarray(
        np.asarray(Wq, np.float32).reshape(NDT, 128, H, 128).transpose(2, 1, 0, 3)
    ).astype(BF16)
    Wk_b = np.ascontiguousarray(
        np.asarray(Wk, np.float32).reshape(NDT, 128, HD).transpose(1, 0, 2)
    ).astype(BF16)
    Wv_b = np.ascontiguousarray(
        np.asarray(Wv, np.float32).reshape(NDT, 128, HD).transpose(1, 0, 2)
    ).astype(BF16)
    Wo_b = np.ascontiguousarray(
        np.asarray(Wo, np.float32).reshape(H, 128, 4, 512).transpose(2, 1, 0, 3)
    ).astype(BF16)
    bo_f = np.ascontiguousarray(np.asarray(bo, np.float32).reshape(1, D))

    inv_freq = np.exp(
        -np.log(np.float32(ROPE_BASE))
        * (np.arange(0, ROPE_DIMS, 2, dtype=np.float32) / np.float32(ROPE_DIMS))
    ).astype(np.float32)

    # window-edge triangle masks, per local key tile s (same for all cores):
    #   s<=3: its newest query block (t=s) sits at the window edge: keep i'<j'
    #   s>=4: its oldest query block (t=s-4) is the causal diagonal: keep i'>=j'
    ar = np.arange(128)
    iq = ar[None, :]   # query within block (columns)
    jk = ar[:, None]   # key within tile (rows)
    m = np.zeros((128, NKT, 128), np.float32)
    for s in range(NKT):
        m[:, s, :] = (iq < jk) if s <= 3 else (iq >= jk)
    masks = np.ascontiguousarray(m.astype(BF16))

    eslide = np.zeros((128, 31), np.float32)
    eslide[:, 15] = 1.0
    eslide = np.ascontiguousarray(eslide.astype(BF16))

    sel16 = np.zeros((16, H, 128), np.float32)
    for h in range(H):
        sel16[h, h, :] = 1.0
    sel16 = np.ascontiguousarray(sel16.astype(BF16))

    perm = np.zeros((ROPE_DIMS, ROPE_DIMS), np.float32)
    perm[(np.arange(ROPE_DIMS) + HALF) % ROPE_DIMS, np.arange(ROPE_DIMS)] = 1.0
    perm = np.ascontiguousarray(perm.astype(BF16))

    in_maps = []
    for c in range(8):
        b, g = divmod(c, 4)
        k_start = 512 * g - 512
        xs = np.zeros((NK, D), np.float32)
        lo = max(0, k_start)
        xs[lo - k_start:] = x[b, lo:k_start + NK]
        xT = np.ascontiguousarray(xs.T).astype(BF16)

        pos = (k_start + np.arange(NK)).astype(np.float32)
        theta = pos[None, :] * inv_freq[:, None]          # [32, NK]
        cos2 = np.ascontiguousarray(
            np.concatenate([np.cos(theta)] * 2, axis=0).astype(np.float32))
        sin2 = np.ascontiguousarray(
            np.concatenate([-np.sin(theta), np.sin(theta)], axis=0).astype(np.float32))

        # kill halo key tiles (absolute tile index < 0) inside the exp
        ebias = np.zeros((128, NKT), np.float32)
        for s in range(NKT):
            if 4 * g - 4 + s < 0:
                ebias[:, s] = -30.0
        ebias = np.ascontiguousarray(ebias)

        in_maps.append({
            "xT": xT, "Wq": Wq_b, "Wk": Wk_b, "Wv": Wv_b, "Wo": Wo_b,
            "bo": bo_f, "cosT": cos2, "sinT": sin2, "masks": masks,
            "ebias": ebias, "eslide": eslide, "sel16": sel16, "perm64": perm,
        })
    return in_maps


def _unshard(results):
    out = np.zeros((B, L, D), np.float32)
    for c in range(8):
        b, g = divmod(c, 4)
        out[b, CHUNK * g:CHUNK * (g + 1)] = results[c]["out"]
    return out


def kernel(x, Wq, Wk, Wv, Wo, bo):
    from concourse.bass_utils import run_bass_kernel_spmd

    nc = _get_program()
    in_maps = _make_in_maps(x, Wq, Wk, Wv, Wo, bo)
    res = run_bass_kernel_spmd(nc, in_maps, core_ids=list(range(8)))
    return _unshard(res.results)


# revision 10
# speedup vs baseline: 1.3214x; 1.0470x over previous
"""Local sliding-window attention block (MQA + partial RoPE) on 8 TRN2 cores.

Sharding: 2 batches x 4 sequence chunks of 512 queries each. Each core
computes q/k/v projections for its chunk (keys include a 512-token halo),
windowed attention (window=512, causal), and the o-projection for its own
query rows — so the host-side unshard is a pure concatenation.

On-chip layout: everything transposed (feature dim on partitions).
  xT[d, pos]  ->  Q^T[dh, q] / K^T[dh, k] (RoPE'd)  ->  S^T[k, q]
  -> exp -> P^T[k, q] (bf16)  -> O^T[dv, q] = V.T-matmul.

Softmax plumbing (all off the GPSIMD/critical path):
  - scores grouped per KEY tile s: one matmul of N = 128*(#query tiles in
    window of s), so 8 S-matmuls per head instead of 20.
  - halo keys (cores at chunk 0) are killed with a per-partition bias AP on
    the Exp activation (exp(0*scale - 30) ~= 0), so no full masks needed;
    only the two window-edge triangles are multiplied in.
  - denominators: vector reduce over the 5 in-window key tiles per query
    tile (bf16), then a [128,16]-selector matmul packs sum_k into row h of
    ONE [16,512] PSUM tile accumulated across all 16 heads. One
    reciprocal_approx_fast on [16,512], one selector matmul per head
    broadcasts 1/d back to [128,512], and a vector multiply normalizes.
  - RoPE half-swap runs as a PE permutation matmul instead of SBUF DMAs.
All matmuls bf16 inputs, fp32 PSUM accumulation.
"""

import numpy as np
import ml_dtypes

BF16 = ml_dtypes.bfloat16

B, L, D = 2, 2048, 2048
H, HD = 16, 128
ROPE_DIMS, HALF = 64, 32
WINDOW = 512
ROPE_BASE = 10000.0
SCALE = HD ** -0.5

CHUNK = 512            # queries per core
NK = 1024              # keys (incl. halo) per core
NQT = CHUNK // 128     # 4 local query tiles
NKT = NK // 128        # 8 local key tiles
NDT = D // 128         # 16 contraction tiles over embedding dim

_PROGRAM = None


def _build_program():
    from contextlib import ExitStack
    import concourse.bass as bass
    import concourse.mybir as mybir
    import concourse.tile as tile
    from concourse import bacc

    fp32 = mybir.dt.float32
    bf16 = mybir.dt.bfloat16
    AF = mybir.ActivationFunctionType

    nc = bacc.Bacc(None, target_bir_lowering=False)

    xT_d = nc.dram_tensor("xT", [D, NK], bf16, kind="ExternalInput")
    wq_d = nc.dram_tensor("Wq", [H, 128, NDT, 128], bf16, kind="ExternalInput")
    wk_d = nc.dram_tensor("Wk", [128, NDT, HD], bf16, kind="ExternalInput")
    wv_d = nc.dram_tensor("Wv", [128, NDT, HD], bf16, kind="ExternalInput")
    wo_d = nc.dram_tensor("Wo", [4, 128, H, 512], bf16, kind="ExternalInput")
    bo_d = nc.dram_tensor("bo", [1, D], fp32, kind="ExternalInput")
    cos_d = nc.dram_tensor("cosT", [ROPE_DIMS, NK], fp32, kind="ExternalInput")
    sin_d = nc.dram_tensor("sinT", [ROPE_DIMS, NK], fp32, kind="ExternalInput")
    msk_d = nc.dram_tensor("masks", [128, NKT, 128], bf16, kind="ExternalInput")
    ebias_d = nc.dram_tensor("ebias", [128, NKT], fp32, kind="ExternalInput")
    esl_d = nc.dram_tensor("eslide", [128, 31], bf16, kind="ExternalInput")
    sel_d = nc.dram_tensor("sel16", [16, H, 128], bf16, kind="ExternalInput")
    prm_d = nc.dram_tensor("perm64", [ROPE_DIMS, ROPE_DIMS], bf16, kind="ExternalInput")
    # blocked [n, t, 128, 512] so each store is one contiguous 256KB region
    out_d = nc.dram_tensor("out", [4, NQT, 128, 512], fp32, kind="ExternalOutput")

    # per key tile s, the query-column range it serves (absolute, in [0,512))
    def qcols(s):
        return max(s - 4, 0) * 128, min(s + 1, 4) * 128

    with tile.TileContext(nc) as tc, ExitStack() as ctx:
        p_const = ctx.enter_context(tc.tile_pool(name="const", bufs=1))
        p_xt = ctx.enter_context(tc.tile_pool(name="xt", bufs=1))
        p_kv = ctx.enter_context(tc.tile_pool(name="kv", bufs=1))
        p_wq = ctx.enter_context(tc.tile_pool(name="wq", bufs=3))
        p_qt = ctx.enter_context(tc.tile_pool(name="qt", bufs=3))
        p_es = ctx.enter_context(tc.tile_pool(name="es", bufs=2))
        p_red = ctx.enter_context(tc.tile_pool(name="red", bufs=4))
        p_tmp = ctx.enter_context(tc.tile_pool(name="tmp", bufs=2))
        p_otn = ctx.enter_context(tc.tile_pool(name="otn", bufs=1))
        p_wo = ctx.enter_context(tc.tile_pool(name="wo", bufs=2))
        p_ob = ctx.enter_context(tc.tile_pool(name="ob", bufs=4))
        ps_perm = ctx.enter_context(
            tc.tile_pool(name="ps_perm", bufs=1, space=bass.MemorySpace.PSUM)
        )

        # ---- persistent loads (ordered so K-proj can start ASAP) ----
        wk_sb = p_const.tile([128, NDT, HD], bf16, tag="wk")
        for i in range(NDT):
            nc.sync.dma_start(out=wk_sb[:, i, :], in_=wk_d[:, i, :])

        xview = xT_d[:].rearrange("(n p) m -> n p m", p=128)
        xt = []
        for i in range(NDT):
            t_ = p_xt.tile([128, NK], bf16, tag=f"xt{i}")
            nc.sync.dma_start(out=t_, in_=xview[i])
            xt.append(t_)

        cos_sb = p_const.tile([ROPE_DIMS, NK], fp32, tag="cos")
        nc.sync.dma_start(out=cos_sb, in_=cos_d[:])
        sin_sb = p_const.tile([ROPE_DIMS, NK], fp32, tag="sin")
        nc.sync.dma_start(out=sin_sb, in_=sin_d[:])
        prm_sb = p_const.tile([ROPE_DIMS, ROPE_DIMS], bf16, tag="perm64")
        nc.sync.dma_start(out=prm_sb, in_=prm_d[:])

        wv_sb = p_const.tile([128, NDT, HD], bf16, tag="wv")
        nc.sync.dma_start(out=wv_sb, in_=wv_d[:])

        msk_sb = p_const.tile([128, NKT, 128], bf16, tag="msk")
        nc.sync.dma_start(out=msk_sb, in_=msk_d[:])
        ebias_sb = p_const.tile([128, NKT], fp32, tag="ebias")
        nc.sync.dma_start(out=ebias_sb, in_=ebias_d[:])
        esl_sb = p_const.tile([128, 31], bf16, tag="eslide")
        nc.sync.dma_start(out=esl_sb, in_=esl_d[:])
        sel_sb = p_const.tile([16, H, 128], bf16, tag="sel16")
        nc.sync.dma_start(out=sel_sb, in_=sel_d[:])

        bias_sb = p_const.tile([128, D], fp32, tag="bias")
        nc.sync.dma_start(
            out=bias_sb, in_=bass.AP(tensor=bo_d, offset=0, ap=[[0, 128], [1, D]])
        )

        def rope(out_bf, ps, cols):
            """out_bf[0:64] = rotary(ps[0:64]); out_bf[64:128] = ps[64:128].

            ps is fp32 PSUM [128, n]; the half-swap (x2 to rows 0:32, x1 to
            rows 32:64) runs as a PE permutation matmul; cos/sin tiles carry
            the duplicated/sign-flipped rows so the rest is partition-aligned
            vector math."""
            n = cols.stop - cols.start
            sb64 = p_tmp.tile([ROPE_DIMS, n], bf16, tag="rope_sb64")
            nc.scalar.copy(sb64, ps[0:ROPE_DIMS])
            psw = ps_perm.tile([ROPE_DIMS, n], fp32, tag="psw")
            nc.tensor.matmul(psw, prm_sb, sb64, start=True, stop=True)
            mcos = p_tmp.tile([ROPE_DIMS, n], fp32, tag="rope_mcos")
            nc.vector.tensor_mul(mcos, ps[0:ROPE_DIMS], cos_sb[:, cols])
            msin = p_tmp.tile([ROPE_DIMS, n], fp32, tag="rope_msin")
            nc.vector.tensor_mul(msin, psw, sin_sb[:, cols])
            nc.vector.tensor_add(out_bf[0:ROPE_DIMS], mcos, msin)
            nc.scalar.copy(out_bf[ROPE_DIMS:HD], ps[ROPE_DIMS:HD])

        # ---- K^T (RoPE'd) and V projections ----
        kt = p_kv.tile([128, NK], bf16, tag="kt")
        v_sb = []
        for s in range(NKT):
            t_ = p_kv.tile([128, HD], bf16, tag=f"v{s}")
            v_sb.append(t_)

        with tc.tile_pool(name="ps_kv", bufs=2, space=bass.MemorySpace.PSUM) as ps_kv:
            for nh in range(NK // 512):
                ps = ps_kv.tile([128, 512], fp32, tag="ps_kv")
                cols = slice(nh * 512, (nh + 1) * 512)
                for dt in range(NDT):
                    nc.tensor.matmul(
                        ps, wk_sb[:, dt, :], xt[dt][:, cols],
                        start=(dt == 0), stop=(dt == NDT - 1),
                    )
                rope(kt[:, cols], ps, cols)

            for s in range(NKT):
                psv = ps_kv.tile([128, HD], fp32, tag="ps_v")
                cols = slice(s * 128, (s + 1) * 128)
                for dt in range(NDT):
                    nc.tensor.matmul(
                        psv, xt[dt][:, cols], wv_sb[:, dt, :],
                        start=(dt == 0), stop=(dt == NDT - 1),
                    )
                nc.scalar.copy(v_sb[s], psv)

        # prefetch the first o-projection weight chunks during attention
        woview = wo_d[:]
        wo_n = []
        for n in range(2):
            t_ = p_wo.tile([128, H, 512], bf16, tag="wo")
            nc.sync.dma_start(out=t_, in_=woview[n])
            wo_n.append(t_)

        # ---- per-head attention ----
        otn = []
        for h in range(H):
            t_ = p_otn.tile([128, CHUNK], bf16, tag=f"otn{h}")
            otn.append(t_)

        with (
            tc.tile_pool(name="ps_q", bufs=2, space=bass.MemorySpace.PSUM) as ps_qp,
            tc.tile_pool(name="ps_s", bufs=2, space=bass.MemorySpace.PSUM) as ps_sp,
            tc.tile_pool(name="ps_o", bufs=2, space=bass.MemorySpace.PSUM) as ps_op,
            tc.tile_pool(name="ps_d", bufs=1, space=bass.MemorySpace.PSUM) as ps_dp,
        ):
            dall = ps_dp.tile([16, CHUNK], fp32, tag="dall")

            def qproj(h):
                wq_h = p_wq.tile([128, NDT, 128], bf16, tag="wq")
                nc.sync.dma_start(out=wq_h, in_=wq_d[h])
                psq = ps_qp.tile([128, CHUNK], fp32, tag="ps_q")
                for dt in range(NDT):
                    nc.tensor.matmul(
                        psq, wq_h[:, dt, :], xt[dt][:, CHUNK:NK],
                        start=(dt == 0), stop=(dt == NDT - 1),
                    )
                qt = p_qt.tile([128, CHUNK], bf16, tag="qt")
                rope(qt, psq, slice(CHUNK, NK))
                return qt

            qt = qproj(0)
            for h in range(H):
                # scores + exp, grouped per key tile
                es = p_es.tile([128, NKT, 512], bf16, tag="es")
                for s in range(NKT):
                    lo, hi = qcols(s)
                    pss = ps_sp.tile([128, 512], fp32, tag="ps_s")
                    nc.tensor.matmul(
                        pss[:, 0:hi - lo],
                        kt[:, s * 128:(s + 1) * 128], qt[:, lo:hi],
                        start=True, stop=True,
                    )
                    nc.scalar.activation(
                        es[:, s, lo:hi], pss[:, 0:hi - lo], AF.Exp,
                        scale=SCALE, bias=ebias_sb[:, s:s + 1],
                    )
                # the 8 window-edge triangles in two strided multiplies:
                # s=0..3 edge at block t=s, s=4..7 edge at block t=s-4 —
                # both sets stride 1024+256 bytes apart in es.
                elo = es[:, 0:4, 0:512].rearrange("p s q -> p (s q)")
                ehi = es[:, 4:NKT, 0:512].rearrange("p s q -> p (s q)")
                st = 512 + 128
                nc.vector.tensor_mul(
                    bass.AP(tensor=elo.tensor, offset=elo.offset,
                            ap=[elo.ap[0], [st, 4], [1, 128]]),
                    bass.AP(tensor=elo.tensor, offset=elo.offset,
                            ap=[elo.ap[0], [st, 4], [1, 128]]),
                    msk_sb[:, 0:4, :],
                )
                nc.vector.tensor_mul(
                    bass.AP(tensor=ehi.tensor, offset=ehi.offset,
                            ap=[ehi.ap[0], [st, 4], [1, 128]]),
                    bass.AP(tensor=ehi.tensor, offset=ehi.offset,
                            ap=[ehi.ap[0], [st, 4], [1, 128]]),
                    msk_sb[:, 4:NKT, :],
                )

                # denominators: contiguous-slice add tree (cheaper than a
                # strided 5-way reduce), then one matmul per head packs
                # sum_k into row h of dall (full-bank accumulation group).
                red = p_red.tile([128, NQT, 128], bf16, tag="red")
                ra = p_red.tile([128, NQT, 128], bf16, tag="ra")
                rb = p_red.tile([128, NQT, 128], bf16, tag="rb")
                for t in range(NQT):
                    tc_ = slice(t * 128, (t + 1) * 128)
                    with nc.allow_low_precision("softmax denom partials in bf16"):
                        nc.vector.tensor_add(
                            ra[:, t, :], es[:, t, tc_], es[:, t + 1, tc_])
                        nc.vector.tensor_add(
                            rb[:, t, :], es[:, t + 2, tc_], es[:, t + 3, tc_])
                        nc.vector.tensor_add(ra[:, t, :], ra[:, t, :], es[:, t + 4, tc_])
                        nc.vector.tensor_add(red[:, t, :], ra[:, t, :], rb[:, t, :])
                # fill the PE while the reductions run: next head's q-proj
                if h + 1 < H:
                    qt = qproj(h + 1)

                nc.tensor.matmul(
                    dall, esl_sb[:, 15 - h:31 - h],
                    red.rearrange("p t q -> p (t q)"),
                    start=(h == 0), stop=(h == H - 1),
                    skip_group_check=True,
                )

                # PV, grouped per key tile: one group per head; the s=0
                # start zeroes the whole otp bank, later tiles accumulate
                # their query-column ranges.
                otp = ps_op.tile([128, CHUNK], fp32, tag="ps_o")
                for s in range(NKT):
                    lo, hi = qcols(s)
                    nc.tensor.matmul(
                        otp[:, lo:hi], v_sb[s], es[:, s, lo:hi],
                        start=(s == 0), stop=(s == NKT - 1),
                        skip_group_check=True,
                    )
                nc.scalar.copy(otn[h], otp)

            # normalize: 1/d once on [16,512], broadcast per head via PE
            dsb = p_const.tile([16, CHUNK], fp32, tag="dsb")
            nc.scalar.copy(dsb, dall)
            rec_f = p_const.tile([16, CHUNK], fp32, tag="rec_f")
            nc.vector.reciprocal_approx_fast(out=rec_f, in_=dsb)
            rec_bf = p_const.tile([16, CHUNK], bf16, tag="rec_bf")
            nc.scalar.copy(rec_bf, rec_f)
            for h in range(H):
                ps_rec = ps_sp.tile([128, 512], fp32, tag="ps_s")
                nc.tensor.matmul(
                    ps_rec, sel_sb[:, h, :], rec_bf, start=True, stop=True
                )
                nc.vector.tensor_mul(otn[h], otn[h], ps_rec)

        # ---- o-projection + bias ----
        with tc.tile_pool(name="ps_out", bufs=4, space=bass.MemorySpace.PSUM) as ps_outp:
            for n in range(D // 512):
                if n >= 2:
                    wo_n[n % 2] = p_wo.tile(
                        [128, H, 512], bf16, tag="wo", name=f"wo{n}"
                    )
                    nc.sync.dma_start(out=wo_n[n % 2], in_=woview[n])
                wo_t = wo_n[n % 2]
                for t in range(NQT):
                    pso = ps_outp.tile([128, 512], fp32, tag="ps_out")
                    for h in range(H):
                        nc.tensor.matmul(
                            pso, otn[h][:, t * 128:(t + 1) * 128], wo_t[:, h, :],
                            start=(h == 0), stop=(h == H - 1),
                        )
                    ob = p_ob.tile([128, 512], fp32, tag="ob")
                    nc.vector.tensor_add(ob, pso, bias_sb[:, n * 512:(n + 1) * 512])
                    nc.sync.dma_start(out=out_d[n, t], in_=ob)

    nc.compile()
    return nc


def _get_program():
    global _PROGRAM
    if _PROGRAM is None:
        _PROGRAM = _build_program()
    return _PROGRAM


def _make_in_maps(x, Wq, Wk, Wv, Wo, bo):
    x = np.asarray(x, np.float32)
    # weights pre-arranged on host so every device DMA is contiguous
    Wq_b = np.ascontiguousarray(
        np.asarray(Wq, np.float32).reshape(NDT, 128, H, 128).transpose(2, 1, 0, 3)
    ).astype(BF16)
    Wk_b = np.ascontiguousarray(
        np.asarray(Wk, np.float32).reshape(NDT, 128, HD).transpose(1, 0, 2)
    ).astype(BF16)
    Wv_b = np.ascontiguousarray(
        np.asarray(Wv, np.float32).reshape(NDT, 128, HD).transpose(1, 0, 2)
    ).astype(BF16)
    Wo_b = np.ascontiguousarray(
        np.asarray(Wo, np.float32).reshape(H, 128, 4, 512).transpose(2, 1, 0, 3)
    ).astype(BF16)
    bo_f = np.ascontiguousarray(np.asarray(bo, np.float32).reshape(1, D))

    inv_freq = np.exp(
        -np.log(np.float32(ROPE_BASE))
        * (np.arange(0, ROPE_DIMS, 2, dtype=np.float32) / np.float32(ROPE_DIMS))
    ).astype(np.float32)

    # window-edge triangle masks, per local key tile s (same for all cores):
    #   s<=3: its newest query block (t=s) sits at the window edge: keep i'<j'
    #   s>=4: its oldest query block (t=s-4) is the causal diagonal: keep i'>=j'
    ar = np.arange(128)
    iq = ar[None, :]   # query within block (columns)
    jk = ar[:, None]   # key within tile (rows)
    m = np.zeros((128, NKT, 128), np.float32)
    for s in range(NKT):
        m[:, s, :] = (iq < jk) if s <= 3 else (iq >= jk)
    masks = np.ascontiguousarray(m.astype(BF16))

    eslide = np.zeros((128, 31), np.float32)
    eslide[:, 15] = 1.0
    eslide = np.ascontiguousarray(eslide.astype(BF16))

    sel16 = np.zeros((16, H, 128), np.float32)
    for h in range(H):
        sel16[h, h, :] = 1.0
    sel16 = np.ascontiguousarray(sel16.astype(BF16))

    perm = np.zeros((ROPE_DIMS, ROPE_DIMS), np.float32)
    perm[(np.arange(ROPE_DIMS) + HALF) % ROPE_DIMS, np.arange(ROPE_DIMS)] = 1.0
    perm = np.ascontiguousarray(perm.astype(BF16))

    in_maps = []
    for c in range(8):
        b, g = divmod(c, 4)
        k_start = 512 * g - 512
        xs = np.zeros((NK, D), np.float32)
        lo = max(0, k_start)
        xs[lo - k_start:] = x[b, lo:k_start + NK]
        xT = np.ascontiguousarray(xs.T).astype(BF16)

        pos = (k_start + np.arange(NK)).astype(np.float32)
        theta = pos[None, :] * inv_freq[:, None]          # [32, NK]
        cos2 = np.ascontiguousarray(
            np.concatenate([np.cos(theta)] * 2, axis=0).astype(np.float32))
        sin2 = np.ascontiguousarray(
            np.concatenate([-np.sin(theta), np.sin(theta)], axis=0).astype(np.float32))

        # kill halo key tiles (absolute tile index < 0) inside the exp
        ebias = np.zeros((128, NKT), np.float32)
        for s in range(NKT):
            if 4 * g - 4 + s < 0:
                ebias[:, s] = -30.0
        ebias = np.ascontiguousarray(ebias)

        in_maps.append({
            "xT": xT, "Wq": Wq_b, "Wk": Wk_b, "Wv": Wv_b, "Wo": Wo_b,
            "bo": bo_f, "cosT": cos2, "sinT": sin2, "masks": masks,
            "ebias": ebias, "eslide": eslide, "sel16": sel16, "perm64": perm,
        })
    return in_maps


def _unshard(results):
    out = np.zeros((B, L, D), np.float32)
    for c in range(8):
        b, g = divmod(c, 4)
        # device layout [n, t, 128, 512] -> [t*128, n*512]
        blk = results[c]["out"].transpose(1, 2, 0, 3).reshape(CHUNK, D)
        out[b, CHUNK * g:CHUNK * (g + 1)] = blk
    return out


def kernel(x, Wq, Wk, Wv, Wo, bo):
    from concourse.bass_utils import run_bass_kernel_spmd

    nc = _get_program()
    in_maps = _make_in_maps(x, Wq, Wk, Wv, Wo, bo)
    res = run_bass_kernel_spmd(nc, in_maps, core_ids=list(range(8)))
    return _unshard(res.results)
